# Initial kernel scaffold
#
"""COTREC GNN message-passing kernel for 8 TRN2 NeuronCores (Bass/Tile SPMD).

Strategy:
- HyperConv (2 sparse layers): edges sorted by destination row, sharded by
  row-range across 8 cores. Per 256-row "supertile", edge messages are fetched
  with 4 bank-split dma_gathers (int16 idx, 4 SWDGE queues). A fused DVE
  tensor_scalar builds a val-scaled one-hot; float32r matmuls accumulate
  psum[112,256] += msg^T @ onehot. PE transposes restore row-major tiles.
  AllGather replicates the updated table between layers.
- SR_IEM attention + SessConv: batch sharded 64 sessions/core; small fp32
  matmuls; AllGather of seq_h; SessConv replicated on every core.
"""
import os
import numpy as np

import concourse.bass as bass
import concourse.bacc as bacc
import concourse.mybir as mybir
import concourse.tile as tile
from concourse import bass_utils
from concourse.masks import make_identity

# ---- problem constants (hardcoded per contract) ----
LAYERS = 2
N_NODE = 100000
EMB = 112
BATCH = 512
SEQ = 50
NNZ = 1600000

NCORES = 8
P = 128
ROWF = 128            # padded row: 128 f32 = 512B
RS = 12544            # rows per core (98 tiles of 128)
NT = RS // P          # 98 tiles per core
STR = 256             # supertile rows
NST = RS // STR       # 49 supertiles
NPAD = NCORES * RS    # 100352 padded table rows
NBANK = 4
BANKROWS = NPAD // NBANK  # 25088
SESS_PER_CORE = BATCH // NCORES  # 64
SESS_NT = SESS_PER_CORE // 2     # 32 tiles, 2 sessions per 128-row tile

F32 = mybir.dt.float32
F32R = mybir.dt.float32r
I16 = mybir.dt.int16
I32 = mybir.dt.int32

TRACE = False
LAST_EXEC_NS = None


# --------------------------------------------------------------------------
# host-side prep: shard + sort edges, build chunked gather/one-hot operands
# --------------------------------------------------------------------------

def _wrap_idx(flat):
    """[n] int16 -> [128, n//16]: idx j -> partition j%16 col j//16, replicated x8."""
    n = flat.shape[0]
    w = flat.reshape(n // 16, 16).T
    return np.tile(w, (8, 1)).astype(np.int16)


def _prep(inputs):
    emb = np.asarray(inputs["embedding"], np.float32)
    rows = np.asarray(inputs["adj_rows"], np.int64)
    cols = np.asarray(inputs["adj_cols"], np.int64)
    vals = np.asarray(inputs["adj_vals"], np.float32)

    # padded fp32 table [NPAD, ROWF]
    table = np.zeros((NPAD, ROWF), np.float32)
    table[:N_NODE, :EMB] = emb

    # sort edges by (core, supertile, bank)
    core = rows // RS
    st = (rows % RS) // STR
    bank = cols // BANKROWS
    key = ((core * NST) + st) * NBANK + bank
    order = np.argsort(key, kind="stable")
    rows_s, cols_s, vals_s, key_s = rows[order], cols[order], vals[order], key[order]

    ngroups = NCORES * NST * NBANK
    counts = np.bincount(key_s, minlength=ngroups)
    CPB = int(np.ceil(counts.max() / P))  # chunks per (st, bank), global max
    GWIDTH = CPB * P                      # padded edges per (st,bank) group
    starts = np.zeros(ngroups + 1, np.int64)
    np.cumsum(counts, out=starts[1:])

    # scatter edges into padded [ngroups, GWIDTH] slots
    pos_in_group = np.arange(len(rows_s)) - starts[key_s]
    slot = key_s * GWIDTH + pos_in_group

    idx16_all = np.empty(ngroups * GWIDTH, np.int16)
    # pad indices: spread pattern over the bank to avoid same-row DMA serialization
    lane = np.arange(ngroups * GWIDTH) % GWIDTH
    idx16_all[:] = ((lane * 37) % BANKROWS).astype(np.int16)
    idx16_all[slot] = (cols_s % BANKROWS).astype(np.int16)

    lrow_all = np.zeros(ngroups * GWIDTH, np.float32)
    lrow_all[slot] = (rows_s % STR).astype(np.float32)
    vals_all = np.zeros(ngroups * GWIDTH, np.float32)
    vals_all[slot] = vals_s

    idx16_all = idx16_all.reshape(NCORES, NST, NBANK, GWIDTH)
    lrow_all = lrow_all.reshape(NCORES, NST, NBANK, CPB, P)
    vals_all = vals_all.reshape(NCORES, NST, NBANK, CPB, P)

    # session-side host prep
    sess_item = np.asarray(inputs["session_item"], np.int64)   # [B, SEQ]
    sess_len = np.asarray(inputs["session_len"], np.float32).reshape(BATCH)
    mask = np.asarray(inputs["mask"], np.float32)              # [B, SEQ]
    W_q = np.asarray(inputs["W_q"], np.float32)
    W_k = np.asarray(inputs["W_k"], np.float32)
    w_sess = np.asarray(inputs["w_sess"], np.float32)
    D = np.asarray(inputs["D"], np.float32)
    A = np.asarray(inputs["A"], np.float32)

    iota = np.tile(np.arange(STR, dtype=np.float32), (P, 1))   # [128, 256]

    in_maps = []
    for c in range(NCORES):
        # wrap idx per (st, bank) gather
        idxw = np.concatenate(
            [_wrap_idx(idx16_all[c, s, b])
             for s in range(NST) for b in range(NBANK)], axis=1)  # [128, NST*NBANK*GWIDTH/16]
        # lrow/vals as [128, nchunks] (lane-major per chunk)
        lrow = lrow_all[c].reshape(NST * NBANK * CPB, P).T.copy()  # [128, NCH]
        valc = vals_all[c].reshape(NST * NBANK * CPB, P).T.copy()

        sl = slice(c * SESS_PER_CORE, (c + 1) * SESS_PER_CORE)
        si = sess_item[sl]        # [64, 50]
        msk = mask[sl]            # [64, 50]
        ln = sess_len[sl]         # [64]
        # session gather idx, 2 sessions per 128-lane tile
        sidx = np.zeros((P, SESS_NT), np.int32)
        for j in range(SESS_NT):
            sidx[0:SEQ, j] = si[2 * j]
            sidx[SEQ:2 * SEQ, j] = si[2 * j + 1]
        # mask replicated across partitions [50, 64*50]
        mask_rep = np.tile(msk.reshape(1, SESS_PER_CORE * SEQ), (SEQ, 1))
        eye2 = np.tile(np.eye(SEQ, dtype=np.float32), (1, 2))   # [50, 100]
        msT = (msk / (ln[:, None] * np.sqrt(np.float32(EMB)))).T.copy()  # [50, 64]
        maskbias = (msk - 1.0) * 1e9

        in_maps.append({
            "table0": table,
            "emb0s": table[c * RS:(c + 1) * RS],
            "idxw": idxw,
            "lrow": lrow,
            "vals": valc,
            "iota": iota,
            "sidx": sidx,
            "mask_rep": np.ascontiguousarray(mask_rep),
            "eye2": eye2,
            "msT": np.ascontiguousarray(msT),
            "mask_sh": np.ascontiguousarray(msk),
            "maskbias": np.ascontiguousarray(maskbias),
            "Wq": W_q, "Wk": W_k,
            "wT1": np.ascontiguousarray(w_sess[0].T),
            "wT2": np.ascontiguousarray(w_sess[1].T),
            "Amat": A,
            "DT": np.ascontiguousarray(D.T),
        })
    return CPB, in_maps


# --------------------------------------------------------------------------
# device program
# --------------------------------------------------------------------------

def _build(CPB, debug=False):
    GW = CPB * P
    NCH = NST * NBANK * CPB
    nc = bacc.Bacc("TRN2", target_bir_lowering=False, debug=False,
                   num_devices=NCORES, num_swdge_queues=4)

    # ---- DRAM I/O ----
    table0 = nc.dram_tensor("table0", [NPAD, ROWF], F32, kind="ExternalInput")
    emb0s = nc.dram_tensor("emb0s", [RS, ROWF], F32, kind="ExternalInput")
    idxw = nc.dram_tensor("idxw", [P, NST * NBANK * GW // 16], I16, kind="ExternalInput")
    lrow_t = nc.dram_tensor("lrow", [P, NCH], F32, kind="ExternalInput")
    vals_t = nc.dram_tensor("vals", [P, NCH], F32, kind="ExternalInput")
    iota_t = nc.dram_tensor("iota", [P, STR], F32, kind="ExternalInput")
    sidx_t = nc.dram_tensor("sidx", [P, SESS_NT], I32, kind="ExternalInput")
    mask_rep_t = nc.dram_tensor("mask_rep", [SEQ, SESS_PER_CORE * SEQ], F32, kind="ExternalInput")
    eye2_t = nc.dram_tensor("eye2", [SEQ, 2 * SEQ], F32, kind="ExternalInput")
    msT_t = nc.dram_tensor("msT", [SEQ, SESS_PER_CORE], F32, kind="ExternalInput")
    mask_sh_t = nc.dram_tensor("mask_sh", [SESS_PER_CORE, SEQ], F32, kind="ExternalInput")
    maskbias_t = nc.dram_tensor("maskbias", [SESS_PER_CORE, SEQ], F32, kind="ExternalInput")
    Wq_t = nc.dram_tensor("Wq", [EMB, EMB], F32, kind="ExternalInput")
    Wk_t = nc.dram_tensor("Wk", [EMB, EMB], F32, kind="ExternalInput")
    wT1_t = nc.dram_tensor("wT1", [EMB, EMB], F32, kind="ExternalInput")
    wT2_t = nc.dram_tensor("wT2", [EMB, EMB], F32, kind="ExternalInput")
    A_t = nc.dram_tensor("Amat", [BATCH, BATCH], F32, kind="ExternalInput")
    DT_t = nc.dram_tensor("DT", [BATCH, BATCH], F32, kind="ExternalInput")

    result = nc.dram_tensor("result", [BATCH, EMB], F32, kind="ExternalOutput")

    # internal DRAM
    emb1_bounce = nc.dram_tensor("emb1_bounce", [RS, ROWF], F32)
    emb1_full = nc.dram_tensor("emb1_full", [NPAD, ROWF], F32, addr_space="Shared")
    item_bounce = nc.dram_tensor("item_bounce", [RS, ROWF], F32)
    cl_full = nc.dram_tensor("cl_full", [1 + NPAD, ROWF], F32, addr_space="Shared")
    seqh_bounce = nc.dram_tensor("seqh_bounce", [SESS_PER_CORE, EMB], F32)
    s0_full = nc.dram_tensor("s0_full", [BATCH, EMB], F32, addr_space="Shared")
    if debug:
        dbg_emb1 = nc.dram_tensor("dbg_emb1", [RS, EMB], F32, kind="ExternalOutput")
        dbg_item = nc.dram_tensor("dbg_item", [RS, EMB], F32, kind="ExternalOutput")
        dbg_seqh = nc.dram_tensor("dbg_seqh", [SESS_PER_CORE, EMB], F32, kind="ExternalOutput")

    RG = [list(range(NCORES))]

    with tile.TileContext(nc) as tc:
        with tc.tile_pool(name="const", bufs=1) as cpool, \
             tc.tile_pool(name="acc", bufs=1) as apool, \
             tc.tile_pool(name="msg", bufs=3) as mpool, \
             tc.tile_pool(name="sel", bufs=6) as spool, \
             tc.tile_pool(name="ev", bufs=4) as epool, \
             tc.tile_pool(name="psA", bufs=3, space="PSUM") as psA, \
             tc.tile_pool(name="psB", bufs=4, space="PSUM") as psB:

            # ---- resident constants ----
            idx_sb = cpool.tile([P, NST * NBANK * GW // 16], I16)
            nc.sync.dma_start(out=idx_sb[:], in_=idxw[:, :])
            lrow_sb = cpool.tile([P, NCH], F32)
            nc.sync.dma_start(out=lrow_sb[:], in_=lrow_t[:, :])
            vals_sb = cpool.tile([P, NCH], F32)
            nc.sync.dma_start(out=vals_sb[:], in_=vals_t[:, :])
            iota_sb = cpool.tile([P, STR], F32)
            nc.sync.dma_start(out=iota_sb[:], in_=iota_t[:, :])
            ident = cpool.tile([P, P], F32)
            make_identity(nc, ident[:])

            acc_sb = apool.tile([P, NT, EMB], F32)

            # ---- one sparse layer ----
            def emit_layer(src_table, layer):
                IDXC = GW // 16  # idx cols per (st,bank) gather
                for s in range(NST):
                    msg = mpool.tile([P, NBANK * CPB, ROWF], F32, tag="msg")
                    for b in range(NBANK):
                        gi = (s * NBANK + b)
                        nc.gpsimd.dma_gather(
                            msg[:, b * CPB:(b + 1) * CPB, :],
                            src_table[b * BANKROWS:(b + 1) * BANKROWS, :],
                            idx_sb[:, gi * IDXC:(gi + 1) * IDXC],
                            GW, GW, ROWF,
                            queue_num=b, single_packet=False)
                    pst = psA.tile([EMB, STR], F32, tag="pst", space="PSUM")
                    nch0 = s * NBANK * CPB
                    ntot = NBANK * CPB
                    for k in range(ntot):
                        ck = nch0 + k
                        sel = spool.tile([P, STR], F32, tag="sel")
                        nc.vector.tensor_scalar(
                            out=sel[:], in0=iota_sb[:],
                            scalar1=lrow_sb[:, ck:ck + 1],
                            scalar2=vals_sb[:, ck:ck + 1],
                            op0=mybir.AluOpType.is_equal,
                            op1=mybir.AluOpType.mult)
                        nc.tensor.matmul(
                            out=pst[:],
                            lhsT=msg[:, k, 0:EMB].bitcast(F32R),
                            rhs=sel[:].bitcast(F32R),
                            start=(k == 0), stop=(k == ntot - 1))
                    # evict: transpose [EMB, 256] -> 2x [128, EMB] row-major tiles
                    tmpT = epool.tile([EMB, STR], F32, tag="tmpT")
                    nc.vector.tensor_copy(out=tmpT[:], in_=pst[:])
                    for h in range(2):
                        t = 2 * s + h
                        ps2 = psB.tile([P, EMB], F32, tag="ps2", space="PSUM")
                        nc.tensor.transpose(
                            out=ps2[:], in_=tmpT[:, h * P:(h + 1) * P], identity=ident[:])
                        if layer == 0:
                            nc.vector.tensor_copy(out=acc_sb[:, t, :], in_=ps2[:])
                            ev = epool.tile([P, EMB], F32, tag="ev")
                            nc.vector.tensor_copy(out=ev[:], in_=ps2[:])
                            nc.sync.dma_start(
                                out=emb1_bounce[t * P:(t + 1) * P, 0:EMB], in_=ev[:])
                            if debug:
                                nc.sync.dma_start(
                                    out=dbg_emb1[t * P:(t + 1) * P, :], in_=ev[:])
                        else:
                            nc.vector.tensor_tensor(
                                out=acc_sb[:, t, :], in0=acc_sb[:, t, :], in1=ps2[:],
                                op=mybir.AluOpType.add)

            emit_layer(table0, 0)
            nc.gpsimd.collective_compute(
                "AllGather", mybir.AluOpType.bypass, replica_groups=RG,
                ins=[emb1_bounce.ap().opt()], outs=[emb1_full.ap().opt()])
            emit_layer(emb1_full, 1)

            # ---- item_emb = (emb0 + acc)/3 -> cl table ----
            zrow = epool.tile([1, ROWF], F32, tag="zrow")
            nc.vector.memset(zrow[:], 0.0)
            nc.sync.dma_start(out=cl_full[0:1, :], in_=zrow[:])
            for t in range(NT):
                e0 = epool.tile([P, EMB], F32, tag="e0")
                nc.sync.dma_start(out=e0[:], in_=emb0s[t * P:(t + 1) * P, 0:EMB])
                it = epool.tile([P, EMB], F32, tag="it")
                nc.vector.tensor_tensor(out=it[:], in0=e0[:], in1=acc_sb[:, t, :],
                                        op=mybir.AluOpType.add)
                nc.vector.tensor_scalar_mul(out=it[:], in0=it[:],
                                            scalar1=1.0 / (LAYERS + 1))
                nc.sync.dma_start(out=item_bounce[t * P:(t + 1) * P, 0:EMB], in_=it[:])
                if debug:
                    nc.sync.dma_start(out=dbg_item[t * P:(t + 1) * P, :], in_=it[:])
            nc.gpsimd.collective_compute(
                "AllGather", mybir.AluOpType.bypass, replica_groups=RG,
                ins=[item_bounce.ap().opt()], outs=[cl_full[1:1 + NPAD, :].opt()])

        # ================= session phase =================
        with tc.tile_pool(name="sconst", bufs=1) as scp, \
             tc.tile_pool(name="swork", bufs=4) as swp, \
             tc.tile_pool(name="spsA", bufs=4, space="PSUM") as spsA, \
             tc.tile_pool(name="spsB", bufs=2, space="PSUM") as spsB:

            ident2 = scp.tile([P, P], F32)
            make_identity(nc, ident2[:])
            sidx_sb = scp.tile([P, SESS_NT], I32)
            nc.sync.dma_start(out=sidx_sb[:], in_=sidx_t[:, :])
            mask_rep_sb = scp.tile([SEQ, SESS_PER_CORE * SEQ], F32)
            nc.sync.dma_start(out=mask_rep_sb[:], in_=mask_rep_t[:, :])
            eye2_sb = scp.tile([SEQ, 2 * SEQ], F32)
            nc.sync.dma_start(out=eye2_sb[:], in_=eye2_t[:, :])
            msT_sb = scp.tile([SEQ, SESS_PER_CORE], F32)
            nc.sync.dma_start(out=msT_sb[:], in_=msT_t[:, :])
            mask_sh_sb = scp.tile([SESS_PER_CORE, SEQ], F32)
            nc.sync.dma_start(out=mask_sh_sb[:], in_=mask_sh_t[:, :])
            maskbias_sb = scp.tile([SESS_PER_CORE, SEQ], F32)
            nc.sync.dma_start(out=maskbias_sb[:], in_=maskbias_t[:, :])
            Wq_sb = scp.tile([EMB, EMB], F32)
            nc.sync.dma_start(out=Wq_sb[:], in_=Wq_t[:, :])
            Wk_sb = scp.tile([EMB, EMB], F32)
            nc.sync.dma_start(out=Wk_sb[:], in_=Wk_t[:, :])
            wT1_sb = scp.tile([EMB, EMB], F32)
            nc.sync.dma_start(out=wT1_sb[:], in_=wT1_t[:, :])
            wT2_sb = scp.tile([EMB, EMB], F32)
            nc.sync.dma_start(out=wT2_sb[:], in_=wT2_t[:, :])
            A_sb = scp.tile([P, 4, BATCH], F32)
            nc.sync.dma_start(out=A_sb[:], in_=A_t.ap().rearrange("(k p) j -> p k j", p=P))
            DT_sb = scp.tile([P, 4, BATCH], F32)
            nc.sync.dma_start(out=DT_sb[:], in_=DT_t.ap().rearrange("(k p) j -> p k j", p=P))

            seq_sb = scp.tile([P, SESS_NT, ROWF], F32)
            seqT_sb = scp.tile([EMB, SESS_NT * P], F32)
            QT_sb = scp.tile([EMB, SESS_NT * P], F32)
            KT_sb = scp.tile([EMB, SESS_NT * P], F32)
            alphaT_sb = scp.tile([SEQ, SESS_PER_CORE], F32)
            betaT_sb = scp.tile([P, SESS_PER_CORE], F32)
            seqh_sb = scp.tile([SESS_PER_CORE, EMB], F32)
            dat_sb = scp.tile([P, 4, BATCH], F32)
            s_sb = scp.tile([P, 4, EMB], F32)
            acc2_sb = scp.tile([P, 4, EMB], F32)

            # DAT = (D@A)^T = A^T @ D^T : lhsT=A chunks, rhs=DT chunks
            for it_ in range(4):
                psd = spsB.tile([P, BATCH], F32, tag="psd", space="PSUM")
                for k in range(4):
                    nc.tensor.matmul(
                        out=psd[:],
                        lhsT=A_sb[:, k, it_ * P:(it_ + 1) * P].bitcast(F32R),
                        rhs=DT_sb[:, k, :].bitcast(F32R),
                        start=(k == 0), stop=(k == 3))
                nc.vector.tensor_copy(out=dat_sb[:, it_, :], in_=psd[:])

            # gather session rows from cl table
            for j in range(SESS_NT):
                nc.gpsimd.indirect_dma_start(
                    out=seq_sb[:, j, :],
                    out_offset=None,
                    in_=cl_full[:, :],
                    in_offset=bass.IndirectOffsetOnAxis(ap=sidx_sb[:, j:j + 1], axis=0))

            # seqT, QT, KT
            for j in range(SESS_NT):
                psT = spsA.tile([EMB, P], F32, tag="psT", space="PSUM")
                nc.tensor.transpose(out=psT[:], in_=seq_sb[:, j, 0:EMB], identity=ident2[:])
                nc.vector.tensor_copy(out=seqT_sb[:, j * P:(j + 1) * P], in_=psT[:])
            for j in range(SESS_NT):
                psq = spsA.tile([EMB, P], F32, tag="psq", space="PSUM")
                nc.tensor.matmul(out=psq[:], lhsT=Wq_sb[:],
                                 rhs=seqT_sb[:, j * P:(j + 1) * P],
                                 start=True, stop=True)
                nc.scalar.activation(out=QT_sb[:, j * P:(j + 1) * P], in_=psq[:],
                                     func=mybir.ActivationFunctionType.Sigmoid)
                psk = spsA.tile([EMB, P], F32, tag="psq", space="PSUM")
                nc.tensor.matmul(out=psk[:], lhsT=Wk_sb[:],
                                 rhs=seqT_sb[:, j * P:(j + 1) * P],
                                 start=True, stop=True)
                nc.scalar.activation(out=KT_sb[:, j * P:(j + 1) * P], in_=psk[:],
                                     func=mybir.ActivationFunctionType.Sigmoid)

            # attention per session pair
            for j in range(SESS_NT):
                psc = spsA.tile([SEQ, 2 * SEQ], F32, tag="psc", space="PSUM")
                for h in range(2):
                    off = j * P + h * SEQ
                    nc.tensor.matmul(out=psc[:, h * SEQ:(h + 1) * SEQ],
                                     lhsT=QT_sb[:, off:off + SEQ],
                                     rhs=KT_sb[:, off:off + SEQ],
                                     start=True, stop=True)
                csig = swp.tile([SEQ, 2 * SEQ], F32, tag="csig")
                nc.scalar.activation(out=csig[:], in_=psc[:],
                                     func=mybir.ActivationFunctionType.Sigmoid)
                tmm = swp.tile([SEQ, 2 * SEQ], F32, tag="tmm")
                nc.vector.tensor_tensor(out=tmm[:], in0=csig[:],
                                        in1=mask_rep_sb[:, j * 2 * SEQ:(j + 1) * 2 * SEQ],
                                        op=mybir.AluOpType.mult)
                r1 = swp.tile([SEQ, 2], F32, tag="r1")
                nc.vector.tensor_reduce(out=r1[:], in_=tmm[:].rearrange("p (a b) -> p a b", a=2),
                                        axis=mybir.AxisListType.X, op=mybir.AluOpType.add)
                nc.vector.tensor_tensor(out=tmm[:], in0=csig[:], in1=eye2_sb[:],
                                        op=mybir.AluOpType.mult)
                dg = swp.tile([SEQ, 2], F32, tag="dg")
                nc.vector.tensor_reduce(out=dg[:], in_=tmm[:].rearrange("p (a b) -> p a b", a=2),
                                        axis=mybir.AxisListType.X, op=mybir.AluOpType.add)
                nc.vector.tensor_tensor(out=r1[:], in0=r1[:], in1=dg[:],
                                        op=mybir.AluOpType.subtract)
                nc.vector.tensor_tensor(out=alphaT_sb[:, 2 * j:2 * j + 2],
                                        in0=r1[:], in1=msT_sb[:, 2 * j:2 * j + 2],
                                        op=mybir.AluOpType.mult)

            # softmax over l (sessions on partitions)
            psa = spsA.tile([SESS_PER_CORE, SEQ], F32, tag="psa", space="PSUM")
            nc.tensor.transpose(out=psa[:], in_=alphaT_sb[:], identity=ident2[:])
            alpha = swp.tile([SESS_PER_CORE, SEQ], F32, tag="alpha")
            nc.vector.tensor_tensor(out=alpha[:], in0=psa[:], in1=mask_sh_sb[:],
                                    op=mybir.AluOpType.mult)
            nc.vector.tensor_tensor(out=alpha[:], in0=alpha[:], in1=maskbias_sb[:],
                                    op=mybir.AluOpType.add)
            mx = swp.tile([SESS_PER_CORE, 1], F32, tag="mx")
            nc.vector.tensor_reduce(out=mx[:], in_=alpha[:],
                                    axis=mybir.AxisListType.X, op=mybir.AluOpType.max)
            nc.vector.tensor_scalar_mul(out=mx[:], in0=mx[:], scalar1=-1.0)
            ex = swp.tile([SESS_PER_CORE, SEQ], F32, tag="ex")
            nc.scalar.activation(out=ex[:], in_=alpha[:],
                                 func=mybir.ActivationFunctionType.Exp,
                                 bias=mx[:, 0:1])
            sm = swp.tile([SESS_PER_CORE, 1], F32, tag="sm")
            nc.vector.tensor_reduce(out=sm[:], in_=ex[:],
                                    axis=mybir.AxisListType.X, op=mybir.AluOpType.add)
            nc.vector.reciprocal(out=sm[:], in_=sm[:])
            beta = swp.tile([SESS_PER_CORE, SEQ], F32, tag="beta")
            nc.vector.tensor_scalar_mul(out=beta[:], in0=beta[:] if False else ex[:],
                                        scalar1=sm[:, 0:1])

            # betaT at partition offsets 0 and 50
            psb2 = spsA.tile([P, SESS_PER_CORE], F32, tag="psb2", space="PSUM")
            nc.tensor.transpose(out=psb2[0:SEQ, :], in_=beta[:], identity=ident2[:])
            nc.tensor.transpose(out=psb2[SEQ:2 * SEQ, :], in_=beta[:], identity=ident2[:])
            nc.vector.tensor_copy(out=betaT_sb[0:2 * SEQ, :], in_=psb2[0:2 * SEQ, :])

            # seq_h
            psh = spsB.tile([SESS_PER_CORE, EMB], F32, tag="psh", space="PSUM")
            for b in range(SESS_PER_CORE):
                j, h = b // 2, b % 2
                nc.tensor.matmul(out=psh[b:b + 1, :],
                                 lhsT=betaT_sb[h * SEQ:(h + 1) * SEQ, b:b + 1],
                                 rhs=seq_sb[h * SEQ:(h + 1) * SEQ, j, 0:EMB],
                                 start=True, stop=True)
            nc.vector.tensor_copy(out=seqh_sb[:], in_=psh[:])
            nc.sync.dma_start(out=seqh_bounce[:, :], in_=seqh_sb[:])
            if debug:
                nc.sync.dma_start(out=dbg_seqh[:, :], in_=seqh_sb[:])
            nc.gpsimd.collective_compute(
                "AllGather", mybir.AluOpType.bypass, replica_groups=RG,
                ins=[seqh_bounce.ap().opt()], outs=[s0_full.ap().opt()])

            # ---- SessConv (replicated on every core) ----
            nc.sync.dma_start(out=s_sb[:], in_=s0_full.ap().rearrange("(k p) d -> p k d", p=P))
            for k in range(4):
                nc.vector.tensor_copy(out=acc2_sb[:, k, :], in_=s_sb[:, k, :])

            sT_sb = scp.tile([EMB, 4 * P], F32)
            t_sb = scp.tile([P, 4, EMB], F32)
            for li, wT in enumerate([wT1_sb, wT2_sb]):
                for k in range(4):
                    pst2 = spsA.tile([EMB, P], F32, tag="pst2", space="PSUM")
                    nc.tensor.transpose(out=pst2[:], in_=s_sb[:, k, :], identity=ident2[:])
                    nc.vector.tensor_copy(out=sT_sb[:, k * P:(k + 1) * P], in_=pst2[:])
                for k in range(4):
                    pt = spsA.tile([P, EMB], F32, tag="pt", space="PSUM")
                    nc.tensor.matmul(out=pt[:], lhsT=sT_sb[:, k * P:(k + 1) * P],
                                     rhs=wT[:], start=True, stop=True)
                    nc.vector.tensor_copy(out=t_sb[:, k, :], in_=pt[:])
                for it_ in range(4):
                    pu = spsA.tile([P, EMB], F32, tag="pu", space="PSUM")
                    for k in range(4):
                        nc.tensor.matmul(out=pu[:],
                                         lhsT=dat_sb[:, k, it_ * P:(it_ + 1) * P],
                                         rhs=t_sb[:, k, :],
                                         start=(k == 0), stop=(k == 3))
                    nc.vector.tensor_copy(out=s_sb[:, it_, :], in_=pu[:])
                    sq = swp.tile([P, EMB], F32, tag="sq")
                    nc.vector.tensor_tensor(out=sq[:], in0=s_sb[:, it_, :],
                                            in1=s_sb[:, it_, :], op=mybir.AluOpType.mult)
                    nr = swp.tile([P, 1], F32, tag="nr")
                    nc.vector.tensor_reduce(out=nr[:], in_=sq[:],
                                            axis=mybir.AxisListType.X,
                                            op=mybir.AluOpType.add)
                    nc.scalar.activation(out=nr[:], in_=nr[:],
                                         func=mybir.ActivationFunctionType.Sqrt)
                    nc.vector.tensor_scalar_max(out=nr[:], in0=nr[:], scalar1=1e-12)
                    nc.vector.reciprocal(out=nr[:], in_=nr[:])
                    nrm = swp.tile([P, EMB], F32, tag="nrm")
                    nc.vector.tensor_scalar_mul(out=nrm[:], in0=s_sb[:, it_, :],
                                                scalar1=nr[:, 0:1])
                    nc.vector.tensor_tensor(out=acc2_sb[:, it_, :], in0=acc2_sb[:, it_, :],
                                            in1=nrm[:], op=mybir.AluOpType.add)

            outt = scp.tile([P, 4, EMB], F32)
            for k in range(4):
                nc.vector.tensor_scalar_mul(out=outt[:, k, :], in0=acc2_sb[:, k, :],
                                            scalar1=1.0 / (LAYERS + 1))
            nc.sync.dma_start(out=result.ap().rearrange("(k p) d -> p k d", p=P), in_=outt[:])

    nc.compile()
    return nc


# --------------------------------------------------------------------------
# entry point
# --------------------------------------------------------------------------

_CACHE = {}


def _get_program(CPB, debug=False):
    key = (CPB, debug)
    if key not in _CACHE:
        _CACHE[key] = _build(CPB, debug)
    return _CACHE[key]


def kernel(**inputs):
    global LAST_EXEC_NS
    CPB, in_maps = _prep(inputs)
    nc = _get_program(CPB, debug=bool(int(os.environ.get("KDEBUG", "0"))))
    trace = TRACE
    if trace:
        try:
            import ntff_shim
            ntff_shim.install()
        except Exception:
            trace = False
    res = bass_utils.run_bass_kernel_spmd(
        nc, in_maps, core_ids=list(range(NCORES)), trace=trace)
    LAST_EXEC_NS = res.exec_time_ns
    kernel.last_results = res.results
    return res.results[0]["result"].astype(np.float32)


# revision 19
# speedup vs baseline: 1.1543x; 1.1543x over previous
"""COTREC GNN message-passing kernel for 8 TRN2 NeuronCores (Bass/Tile SPMD).

Strategy:
- HyperConv (2 sparse layers): edges sorted by destination row, sharded by
  row-range across 8 cores. Per 256-row "supertile", edge messages are fetched
  with 4 bank-split dma_gathers (int16 idx, 4 SWDGE queues). A fused DVE
  tensor_scalar builds a val-scaled one-hot; float32r matmuls accumulate
  psum[112,256] += msg^T @ onehot. PE transposes restore row-major tiles.
  AllGather replicates the updated table between layers.
- SR_IEM attention + SessConv: batch sharded 64 sessions/core; small fp32
  matmuls; AllGather of seq_h; SessConv replicated on every core.
"""
import os
import numpy as np
import ml_dtypes

import concourse.bass as bass
import concourse.bacc as bacc
import concourse.mybir as mybir
import concourse.tile as tile
from concourse import bass_utils
from concourse.masks import make_identity

# ---- problem constants (hardcoded per contract) ----
LAYERS = 2
N_NODE = 100000
EMB = 112
BATCH = 512
SEQ = 50
NNZ = 1600000

NCORES = 8
P = 128
ROWF = 128            # padded row: 128 f32 = 512B
RS = 12544            # rows per core (98 tiles of 128)
NT = RS // P          # 98 tiles per core
STR = 128             # supertile rows (= one output tile)
NST = RS // STR       # 98 supertiles
NPAD = NCORES * RS    # 100352 padded table rows
NBANK = 4
BANKROWS = NPAD // NBANK  # 25088
SESS_PER_CORE = BATCH // NCORES  # 64
SESS_NT = SESS_PER_CORE // 2     # 32 tiles, 2 sessions per 128-row tile

F32 = mybir.dt.float32
F32R = mybir.dt.float32r
BF16 = mybir.dt.bfloat16
I16 = mybir.dt.int16
I32 = mybir.dt.int32

TRACE = False
LAST_EXEC_NS = None


# --------------------------------------------------------------------------
# host-side prep: shard + sort edges, build chunked gather/one-hot operands
# --------------------------------------------------------------------------

def _wrap_idx(flat):
    """[n] int16 -> [128, n//16]: idx j -> partition j%16 col j//16, replicated x8."""
    n = flat.shape[0]
    w = flat.reshape(n // 16, 16).T
    return np.tile(w, (8, 1)).astype(np.int16)


def _prep(inputs):
    emb = np.asarray(inputs["embedding"], np.float32)
    rows = np.asarray(inputs["adj_rows"], np.int64)
    cols = np.asarray(inputs["adj_cols"], np.int64)
    vals = np.asarray(inputs["adj_vals"], np.float32)

    # padded tables: bf16 for the gather path, fp32 for the acc init
    table = np.zeros((NPAD, ROWF), np.float32)
    table[:N_NODE, :EMB] = emb
    table_bf = table.astype(ml_dtypes.bfloat16)

    # sort edges by (core, supertile, bank)
    core = rows // RS
    st = (rows % RS) // STR
    bank = cols // BANKROWS
    key = ((core * NST) + st) * NBANK + bank
    order = np.argsort(key, kind="stable")
    rows_s, cols_s, vals_s, key_s = rows[order], cols[order], vals[order], key[order]

    ngroups = NCORES * NST * NBANK
    counts = np.bincount(key_s, minlength=ngroups)
    CPB = int(np.ceil(counts.max() / P))  # chunks per (st, bank), global max
    GWIDTH = CPB * P                      # padded edges per (st,bank) group
    starts = np.zeros(ngroups + 1, np.int64)
    np.cumsum(counts, out=starts[1:])

    # scatter edges into padded [ngroups, GWIDTH] slots
    pos_in_group = np.arange(len(rows_s)) - starts[key_s]
    slot = key_s * GWIDTH + pos_in_group

    idx16_all = np.empty(ngroups * GWIDTH, np.int16)
    # pad indices: spread pattern over the bank to avoid same-row DMA serialization
    lane = np.arange(ngroups * GWIDTH) % GWIDTH
    idx16_all[:] = ((lane * 37) % BANKROWS).astype(np.int16)
    idx16_all[slot] = (cols_s % BANKROWS).astype(np.int16)

    lrow_all = np.zeros(ngroups * GWIDTH, np.float32)
    lrow_all[slot] = (rows_s % STR).astype(np.float32)
    vals_all = np.zeros(ngroups * GWIDTH, np.float32)
    vals_all[slot] = vals_s

    idx16_all = idx16_all.reshape(NCORES, NST, NBANK, GWIDTH)
    lrow_all = lrow_all.reshape(NCORES, NST, NBANK, CPB, P)
    vals_all = vals_all.reshape(NCORES, NST, NBANK, CPB, P)

    # session-side host prep
    sess_item = np.asarray(inputs["session_item"], np.int64)   # [B, SEQ]
    sess_len = np.asarray(inputs["session_len"], np.float32).reshape(BATCH)
    mask = np.asarray(inputs["mask"], np.float32)              # [B, SEQ]
    W_q = np.asarray(inputs["W_q"], np.float32)
    W_k = np.asarray(inputs["W_k"], np.float32)
    w_sess = np.asarray(inputs["w_sess"], np.float32)
    D = np.asarray(inputs["D"], np.float32)
    A = np.asarray(inputs["A"], np.float32)

    iota = np.tile(np.arange(STR, dtype=np.float32), (P, 1)).astype(ml_dtypes.bfloat16)

    in_maps = []
    for c in range(NCORES):
        # wrap idx per (st, bank) gather
        idxw = np.concatenate(
            [_wrap_idx(idx16_all[c, s, b])
             for s in range(NST) for b in range(NBANK)], axis=1)  # [128, NST*NBANK*GWIDTH/16]
        # lrow/vals as [128, nchunks] (lane-major per chunk)
        lrow = lrow_all[c].reshape(NST * NBANK * CPB, P).T.copy()
        valc = vals_all[c].reshape(NST * NBANK * CPB, P).T.copy()

        sl = slice(c * SESS_PER_CORE, (c + 1) * SESS_PER_CORE)
        si = sess_item[sl]        # [64, 50]
        msk = mask[sl]            # [64, 50]
        ln = sess_len[sl]         # [64]
        # session gather idx, 2 sessions per 128-lane tile
        sidx = np.zeros((P, SESS_NT), np.int32)
        for j in range(SESS_NT):
            sidx[0:SEQ, j] = si[2 * j]
            sidx[64:64 + SEQ, j] = si[2 * j + 1]
        # mask replicated across partitions [50, 64*50]
        mask_rep = np.tile(msk.reshape(1, SESS_PER_CORE * SEQ), (SEQ, 1))
        eye2 = np.tile(np.eye(SEQ, dtype=np.float32), (1, 2))   # [50, 100]
        msT = (msk / (ln[:, None] * np.sqrt(np.float32(EMB)))).T.copy()  # [50, 64]
        maskbias = (msk - 1.0) * 1e9

        in_maps.append({
            "table0": table_bf,
            "emb0s": table[c * RS:(c + 1) * RS],
            "idxw": idxw,
            "lrow": lrow,
            "vals": valc,
            "iota": iota,
            "sidx": sidx,
            "mask_rep": np.ascontiguousarray(mask_rep),
            "eye2": eye2,
            "msT": np.ascontiguousarray(msT),
            "mask_sh": np.ascontiguousarray(msk),
            "maskbias": np.ascontiguousarray(maskbias),
            "Wq": W_q, "Wk": W_k,
            "wT1": np.ascontiguousarray(w_sess[0].T),
            "wT2": np.ascontiguousarray(w_sess[1].T),
            "Amat": A,
            "DT": np.ascontiguousarray(D.T),
        })
    return CPB, in_maps


# --------------------------------------------------------------------------
# device program
# --------------------------------------------------------------------------

def _build(CPB, debug=False):
    GW = CPB * P
    NCH = NST * NBANK * CPB
    nc = bacc.Bacc("TRN2", target_bir_lowering=False, debug=False,
                   num_devices=NCORES, num_swdge_queues=4)

    # ---- DRAM I/O ----
    table0 = nc.dram_tensor("table0", [NPAD, ROWF], BF16, kind="ExternalInput")
    emb0s = nc.dram_tensor("emb0s", [RS, ROWF], F32, kind="ExternalInput")
    idxw = nc.dram_tensor("idxw", [P, NST * NBANK * GW // 16], I16, kind="ExternalInput")
    lrow_t = nc.dram_tensor("lrow", [P, NCH], F32, kind="ExternalInput")
    vals_t = nc.dram_tensor("vals", [P, NCH], F32, kind="ExternalInput")
    iota_t = nc.dram_tensor("iota", [P, STR], BF16, kind="ExternalInput")
    sidx_t = nc.dram_tensor("sidx", [P, SESS_NT], I32, kind="ExternalInput")
    mask_rep_t = nc.dram_tensor("mask_rep", [SEQ, SESS_PER_CORE * SEQ], F32, kind="ExternalInput")
    eye2_t = nc.dram_tensor("eye2", [SEQ, 2 * SEQ], F32, kind="ExternalInput")
    msT_t = nc.dram_tensor("msT", [SEQ, SESS_PER_CORE], F32, kind="ExternalInput")
    mask_sh_t = nc.dram_tensor("mask_sh", [SESS_PER_CORE, SEQ], F32, kind="ExternalInput")
    maskbias_t = nc.dram_tensor("maskbias", [SESS_PER_CORE, SEQ], F32, kind="ExternalInput")
    Wq_t = nc.dram_tensor("Wq", [EMB, EMB], F32, kind="ExternalInput")
    Wk_t = nc.dram_tensor("Wk", [EMB, EMB], F32, kind="ExternalInput")
    wT1_t = nc.dram_tensor("wT1", [EMB, EMB], F32, kind="ExternalInput")
    wT2_t = nc.dram_tensor("wT2", [EMB, EMB], F32, kind="ExternalInput")
    A_t = nc.dram_tensor("Amat", [BATCH, BATCH], F32R, kind="ExternalInput")
    DT_t = nc.dram_tensor("DT", [BATCH, BATCH], F32R, kind="ExternalInput")

    result = nc.dram_tensor("result", [BATCH, EMB], F32, kind="ExternalOutput")

    # internal DRAM
    emb1_bounce = nc.dram_tensor("emb1_bounce", [RS, ROWF], BF16)
    emb1_full = nc.dram_tensor("emb1_full", [NPAD, ROWF], BF16, addr_space="Shared")
    item_bounce = nc.dram_tensor("item_bounce", [RS, ROWF], F32)
    cl_full = nc.dram_tensor("cl_full", [1 + NPAD, ROWF], F32, addr_space="Shared")
    seqh_bounce = nc.dram_tensor("seqh_bounce", [SESS_PER_CORE, EMB], F32)
    s0_full = nc.dram_tensor("s0_full", [BATCH, EMB], F32, addr_space="Shared")
    if debug:
        dbg_emb1 = nc.dram_tensor("dbg_emb1", [RS, EMB], F32, kind="ExternalOutput")
        dbg_item = nc.dram_tensor("dbg_item", [RS, EMB], F32, kind="ExternalOutput")
        dbg_seqh = nc.dram_tensor("dbg_seqh", [SESS_PER_CORE, EMB], F32, kind="ExternalOutput")

    RG = [list(range(NCORES))]

    with tile.TileContext(nc) as tc:
        with tc.tile_pool(name="const", bufs=1) as cpool, \
             tc.tile_pool(name="acc", bufs=1) as apool, \
             tc.tile_pool(name="msg", bufs=4) as mpool, \
             tc.tile_pool(name="sel", bufs=16) as spool, \
             tc.tile_pool(name="ev", bufs=4) as epool, \
             tc.tile_pool(name="psA", bufs=4, space="PSUM") as psA, \
             tc.tile_pool(name="psB", bufs=4, space="PSUM") as psB:

            # ---- resident constants ----
            idx_sb = cpool.tile([P, NST * NBANK * GW // 16], I16)
            nc.sync.dma_start(out=idx_sb[:], in_=idxw[:, :])
            lrow_sb = cpool.tile([P, NCH], F32)
            nc.sync.dma_start(out=lrow_sb[:], in_=lrow_t[:, :])
            vals_sb = cpool.tile([P, NCH], F32)
            nc.sync.dma_start(out=vals_sb[:], in_=vals_t[:, :])
            iota_sb = cpool.tile([P, STR], BF16)
            nc.sync.dma_start(out=iota_sb[:], in_=iota_t[:, :])
            ident = cpool.tile([P, P], F32)
            make_identity(nc, ident[:])

            acc_sb = apool.tile([P, NT, EMB], F32)

            # ---- one sparse layer ----
            def emit_layer(src_table, layer):
                IDXC = GW // 16  # idx cols per (st,bank) gather
                for s in range(NST):
                    msg = mpool.tile([P, NBANK * CPB, ROWF], BF16, tag="msg")
                    for b in range(NBANK):
                        gi = (s * NBANK + b)
                        nc.gpsimd.dma_gather(
                            msg[:, b * CPB:(b + 1) * CPB, :],
                            src_table[b * BANKROWS:(b + 1) * BANKROWS, :],
                            idx_sb[:, gi * IDXC:(gi + 1) * IDXC],
                            GW, GW, ROWF,
                            queue_num=b, single_packet=False)
                    pst = psA.tile([P, EMB], F32, tag="pst", space="PSUM")
                    nch0 = s * NBANK * CPB
                    ntot = NBANK * CPB
                    for k in range(ntot):
                        ck = nch0 + k
                        sel = spool.tile([P, STR], BF16, tag="sel")
                        nc.vector.tensor_scalar(
                            out=sel[:], in0=iota_sb[:],
                            scalar1=lrow_sb[:, ck:ck + 1],
                            scalar2=vals_sb[:, ck:ck + 1],
                            op0=mybir.AluOpType.is_equal,
                            op1=mybir.AluOpType.mult)
                        nc.tensor.matmul(
                            out=pst[:],
                            lhsT=sel[:],
                            rhs=msg[:, k, 0:EMB],
                            start=(k == 0), stop=(k == ntot - 1))
                    # evict row-major tile directly
                    t = s
                    if layer == 0:
                        nc.scalar.copy(out=acc_sb[:, t, :], in_=pst[:])
                        ev = epool.tile([P, EMB], BF16, tag="ev")
                        nc.scalar.copy(out=ev[:], in_=pst[:])
                        nc.sync.dma_start(
                            out=emb1_bounce[t * P:(t + 1) * P, 0:EMB], in_=ev[:])
                        if debug:
                            evd = epool.tile([P, EMB], F32, tag="evd")
                            nc.scalar.copy(out=evd[:], in_=pst[:])
                            nc.sync.dma_start(
                                out=dbg_emb1[t * P:(t + 1) * P, :], in_=evd[:])
                    else:
                        nc.vector.tensor_tensor(
                            out=acc_sb[:, t, :], in0=acc_sb[:, t, :], in1=pst[:],
                            op=mybir.AluOpType.add)

            emit_layer(table0, 0)
            nc.gpsimd.collective_compute(
                "AllGather", mybir.AluOpType.bypass, replica_groups=RG,
                ins=[emb1_bounce.ap().opt()], outs=[emb1_full.ap().opt()])
            emit_layer(emb1_full, 1)

            # ---- item_emb = (emb0 + acc)/3 -> cl table ----
            zrow = epool.tile([1, ROWF], F32, tag="zrow")
            nc.vector.memset(zrow[:], 0.0)
            nc.sync.dma_start(out=cl_full[0:1, :], in_=zrow[:])
            for t in range(NT):
                e0 = epool.tile([P, EMB], F32, tag="e0")
                nc.sync.dma_start(out=e0[:], in_=emb0s[t * P:(t + 1) * P, 0:EMB])
                it = epool.tile([P, EMB], F32, tag="it")
                nc.vector.tensor_tensor(out=it[:], in0=e0[:], in1=acc_sb[:, t, :],
                                        op=mybir.AluOpType.add)
                nc.vector.tensor_scalar_mul(out=it[:], in0=it[:],
                                            scalar1=1.0 / (LAYERS + 1))
                nc.sync.dma_start(out=item_bounce[t * P:(t + 1) * P, 0:EMB], in_=it[:])
                if debug:
                    nc.sync.dma_start(out=dbg_item[t * P:(t + 1) * P, :], in_=it[:])
            nc.gpsimd.collective_compute(
                "AllGather", mybir.AluOpType.bypass, replica_groups=RG,
                ins=[item_bounce.ap().opt()], outs=[cl_full[1:1 + NPAD, :].opt()])

        # ================= session phase =================
        with tc.tile_pool(name="sconst", bufs=1) as scp, \
             tc.tile_pool(name="swork", bufs=4) as swp, \
             tc.tile_pool(name="spsA", bufs=2, space="PSUM") as spsA, \
             tc.tile_pool(name="spsB", bufs=2, space="PSUM") as spsB:

            ident2 = scp.tile([P, P], F32)
            make_identity(nc, ident2[:])
            sidx_sb = scp.tile([P, SESS_NT], I32)
            nc.sync.dma_start(out=sidx_sb[:], in_=sidx_t[:, :])
            mask_rep_sb = scp.tile([SEQ, SESS_PER_CORE * SEQ], F32)
            nc.sync.dma_start(out=mask_rep_sb[:], in_=mask_rep_t[:, :])
            eye2_sb = scp.tile([SEQ, 2 * SEQ], F32)
            nc.sync.dma_start(out=eye2_sb[:], in_=eye2_t[:, :])
            msT_sb = scp.tile([SEQ, SESS_PER_CORE], F32)
            nc.sync.dma_start(out=msT_sb[:], in_=msT_t[:, :])
            mask_sh_sb = scp.tile([SESS_PER_CORE, SEQ], F32)
            nc.sync.dma_start(out=mask_sh_sb[:], in_=mask_sh_t[:, :])
            maskbias_sb = scp.tile([SESS_PER_CORE, SEQ], F32)
            nc.sync.dma_start(out=maskbias_sb[:], in_=maskbias_t[:, :])
            Wq_sb = scp.tile([EMB, EMB], F32)
            nc.sync.dma_start(out=Wq_sb[:], in_=Wq_t[:, :])
            Wk_sb = scp.tile([EMB, EMB], F32)
            nc.sync.dma_start(out=Wk_sb[:], in_=Wk_t[:, :])
            wT1_sb = scp.tile([EMB, EMB], F32)
            nc.sync.dma_start(out=wT1_sb[:], in_=wT1_t[:, :])
            wT2_sb = scp.tile([EMB, EMB], F32)
            nc.sync.dma_start(out=wT2_sb[:], in_=wT2_t[:, :])
            A_sb = scp.tile([P, 4, BATCH], F32R)
            DT_sb = scp.tile([P, 4, BATCH], F32R)
            for k in range(4):
                nc.sync.dma_start(out=A_sb[:, k, :], in_=A_t[k * P:(k + 1) * P, :])
                nc.sync.dma_start(out=DT_sb[:, k, :], in_=DT_t[k * P:(k + 1) * P, :])

            seq_sb = scp.tile([P, SESS_NT, ROWF], F32)
            seqT_sb = scp.tile([EMB, SESS_NT * P], F32)
            QT_sb = scp.tile([EMB, SESS_NT * P], F32)
            KT_sb = scp.tile([EMB, SESS_NT * P], F32)
            alphaT_sb = scp.tile([SEQ, SESS_PER_CORE], F32)
            betaT_sb = scp.tile([P, SESS_PER_CORE], F32)
            seqh_sb = scp.tile([SESS_PER_CORE, EMB], F32)
            dat_sb = scp.tile([P, 4, BATCH], F32)
            s_sb = scp.tile([P, 4, EMB], F32)
            acc2_sb = scp.tile([P, 4, EMB], F32)

            # DAT = (D@A)^T = A^T @ D^T : lhsT=A chunks, rhs=DT chunks
            for it_ in range(4):
                psd = spsB.tile([P, BATCH], F32, tag="b", space="PSUM")
                for k in range(4):
                    nc.tensor.matmul(
                        out=psd[:],
                        lhsT=A_sb[:, k, it_ * P:(it_ + 1) * P],
                        rhs=DT_sb[:, k, :],
                        start=(k == 0), stop=(k == 3))
                nc.vector.tensor_copy(out=dat_sb[:, it_, :], in_=psd[:])

            # gather session rows from cl table
            for j in range(SESS_NT):
                nc.gpsimd.indirect_dma_start(
                    out=seq_sb[:, j, :],
                    out_offset=None,
                    in_=cl_full[:, :],
                    in_offset=bass.IndirectOffsetOnAxis(ap=sidx_sb[:, j:j + 1], axis=0))

            # seqT, QT, KT
            for j in range(SESS_NT):
                psT = spsA.tile([EMB, P], F32, tag="a", space="PSUM")
                nc.tensor.transpose(out=psT[:], in_=seq_sb[:, j, 0:EMB], identity=ident2[:])
                nc.vector.tensor_copy(out=seqT_sb[:, j * P:(j + 1) * P], in_=psT[:])
            for j in range(SESS_NT):
                psq = spsA.tile([EMB, P], F32, tag="a", space="PSUM")
                nc.tensor.matmul(out=psq[:], lhsT=Wq_sb[:],
                                 rhs=seqT_sb[:, j * P:(j + 1) * P],
                                 start=True, stop=True)
                nc.scalar.activation(out=QT_sb[:, j * P:(j + 1) * P], in_=psq[:],
                                     func=mybir.ActivationFunctionType.Sigmoid)
                psk = spsA.tile([EMB, P], F32, tag="a", space="PSUM")
                nc.tensor.matmul(out=psk[:], lhsT=Wk_sb[:],
                                 rhs=seqT_sb[:, j * P:(j + 1) * P],
                                 start=True, stop=True)
                nc.scalar.activation(out=KT_sb[:, j * P:(j + 1) * P], in_=psk[:],
                                     func=mybir.ActivationFunctionType.Sigmoid)

            # attention per session pair
            for j in range(SESS_NT):
                psc = spsA.tile([SEQ, 2 * SEQ], F32, tag="a", space="PSUM")
                for h in range(2):
                    off = j * P + h * 64
                    nc.tensor.matmul(out=psc[:, h * SEQ:(h + 1) * SEQ],
                                     lhsT=QT_sb[:, off:off + SEQ],
                                     rhs=KT_sb[:, off:off + SEQ],
                                     start=True, stop=True)
                csig = swp.tile([SEQ, 2 * SEQ], F32, tag="csig")
                nc.scalar.activation(out=csig[:], in_=psc[:],
                                     func=mybir.ActivationFunctionType.Sigmoid)
                tmm = swp.tile([SEQ, 2 * SEQ], F32, tag="tmm")
                nc.vector.tensor_tensor(out=tmm[:], in0=csig[:],
                                        in1=mask_rep_sb[:, j * 2 * SEQ:(j + 1) * 2 * SEQ],
                                        op=mybir.AluOpType.mult)
                r1 = swp.tile([SEQ, 2], F32, tag="r1")
                nc.vector.tensor_reduce(out=r1[:], in_=tmm[:].rearrange("p (a b) -> p a b", a=2),
                                        axis=mybir.AxisListType.X, op=mybir.AluOpType.add)
                nc.vector.tensor_tensor(out=tmm[:], in0=csig[:], in1=eye2_sb[:],
                                        op=mybir.AluOpType.mult)
                dg = swp.tile([SEQ, 2], F32, tag="dg")
                nc.vector.tensor_reduce(out=dg[:], in_=tmm[:].rearrange("p (a b) -> p a b", a=2),
                                        axis=mybir.AxisListType.X, op=mybir.AluOpType.add)
                nc.vector.tensor_tensor(out=r1[:], in0=r1[:], in1=dg[:],
                                        op=mybir.AluOpType.subtract)
                nc.vector.tensor_tensor(out=alphaT_sb[:, 2 * j:2 * j + 2],
                                        in0=r1[:], in1=msT_sb[:, 2 * j:2 * j + 2],
                                        op=mybir.AluOpType.mult)

            # softmax over l (sessions on partitions)
            psa = spsA.tile([SESS_PER_CORE, SEQ], F32, tag="a", space="PSUM")
            nc.tensor.transpose(out=psa[:], in_=alphaT_sb[:], identity=ident2[0:SEQ, 0:SEQ])
            alpha = swp.tile([SESS_PER_CORE, SEQ], F32, tag="alpha")
            nc.vector.tensor_tensor(out=alpha[:], in0=psa[:], in1=mask_sh_sb[:],
                                    op=mybir.AluOpType.mult)
            nc.vector.tensor_tensor(out=alpha[:], in0=alpha[:], in1=maskbias_sb[:],
                                    op=mybir.AluOpType.add)
            mx = swp.tile([SESS_PER_CORE, 1], F32, tag="mx")
            nc.vector.tensor_reduce(out=mx[:], in_=alpha[:],
                                    axis=mybir.AxisListType.X, op=mybir.AluOpType.max)
            nc.vector.tensor_scalar_mul(out=mx[:], in0=mx[:], scalar1=-1.0)
            ex = swp.tile([SESS_PER_CORE, SEQ], F32, tag="ex")
            nc.scalar.activation(out=ex[:], in_=alpha[:],
                                 func=mybir.ActivationFunctionType.Exp,
                                 bias=mx[:, 0:1])
            sm = swp.tile([SESS_PER_CORE, 1], F32, tag="sm")
            nc.vector.tensor_reduce(out=sm[:], in_=ex[:],
                                    axis=mybir.AxisListType.X, op=mybir.AluOpType.add)
            nc.vector.reciprocal(out=sm[:], in_=sm[:])
            beta = swp.tile([SESS_PER_CORE, SEQ], F32, tag="beta")
            nc.vector.tensor_scalar_mul(out=beta[:], in0=ex[:], scalar1=sm[:, 0:1])

            # betaT on partitions 0-49 (direct) and 64-113 (via zero-padded input,
            # since matmul psum outputs must start at partition 0)
            psb2 = spsA.tile([SEQ, SESS_PER_CORE], F32, tag="a", space="PSUM")
            nc.tensor.transpose(out=psb2[:], in_=beta[:],
                                identity=ident2[0:SESS_PER_CORE, 0:SESS_PER_CORE])
            nc.vector.tensor_copy(out=betaT_sb[0:SEQ, :], in_=psb2[:])
            betap = swp.tile([SESS_PER_CORE, 64 + SEQ], F32, tag="betap")
            nc.vector.memset(betap[:, 0:64], 0.0)
            nc.vector.tensor_copy(out=betap[:, 64:64 + SEQ], in_=beta[:])
            psb3 = spsA.tile([64 + SEQ, SESS_PER_CORE], F32, tag="a", space="PSUM")
            nc.tensor.transpose(out=psb3[:], in_=betap[:],
                                identity=ident2[0:SESS_PER_CORE, 0:SESS_PER_CORE])
            nc.vector.tensor_copy(out=betaT_sb[64:64 + SEQ, :], in_=psb3[64:64 + SEQ, :])

            # beta pattern bp3[p, j, b]: nonzero only for b in {2j, 2j+1} at the
            # session's lanes; built with 2 strided copies over a zeroed tile.
            bp3 = scp.tile([P, SESS_NT, SESS_PER_CORE], F32)
            nc.vector.memset(bp3[:], 0.0)
            bp3f = bp3[:].rearrange("p a b -> p (a b)")
            W2 = SESS_PER_CORE + 2  # stride 66 hits (j, 2j)
            nc.vector.tensor_copy(
                out=bp3f[0:SEQ, 0:SESS_NT * SESS_PER_CORE:W2],
                in_=betaT_sb[0:SEQ, 0:SESS_PER_CORE:2])
            nc.vector.tensor_copy(
                out=bp3f[64:64 + SEQ, 1:SESS_NT * SESS_PER_CORE:W2],
                in_=betaT_sb[64:64 + SEQ, 1:SESS_PER_CORE:2])

            # seq_h: accumulate over the 32 session tiles into one psum [64, EMB]
            psh = spsB.tile([SESS_PER_CORE, EMB], F32, tag="b", space="PSUM")
            for j in range(SESS_NT):
                nc.tensor.matmul(out=psh[:],
                                 lhsT=bp3[:, j, :],
                                 rhs=seq_sb[:, j, 0:EMB],
                                 start=(j == 0), stop=(j == SESS_NT - 1))
            nc.vector.tensor_copy(out=seqh_sb[:], in_=psh[:])
            nc.sync.dma_start(out=seqh_bounce[:, :], in_=seqh_sb[:])
            if debug:
                nc.sync.dma_start(out=dbg_seqh[:, :], in_=seqh_sb[:])
            nc.gpsimd.collective_compute(
                "AllGather", mybir.AluOpType.bypass, replica_groups=RG,
                ins=[seqh_bounce.ap().opt()], outs=[s0_full.ap().opt()])

            # ---- SessConv (replicated on every core) ----
            for k in range(4):
                nc.sync.dma_start(out=s_sb[:, k, :], in_=s0_full[k * P:(k + 1) * P, :])
                nc.vector.tensor_copy(out=acc2_sb[:, k, :], in_=s_sb[:, k, :])

            sT_sb = scp.tile([EMB, 4 * P], F32)
            t_sb = scp.tile([P, 4, EMB], F32)
            for li, wT in enumerate([wT1_sb, wT2_sb]):
                for k in range(4):
                    pst2 = spsA.tile([EMB, P], F32, tag="a", space="PSUM")
                    nc.tensor.transpose(out=pst2[:], in_=s_sb[:, k, :], identity=ident2[:])
                    nc.vector.tensor_copy(out=sT_sb[:, k * P:(k + 1) * P], in_=pst2[:])
                for k in range(4):
                    pt = spsA.tile([P, EMB], F32, tag="a", space="PSUM")
                    nc.tensor.matmul(out=pt[:], lhsT=sT_sb[:, k * P:(k + 1) * P],
                                     rhs=wT[:], start=True, stop=True)
                    nc.vector.tensor_copy(out=t_sb[:, k, :], in_=pt[:])
                for it_ in range(4):
                    pu = spsA.tile([P, EMB], F32, tag="a", space="PSUM")
                    for k in range(4):
                        nc.tensor.matmul(out=pu[:],
                                         lhsT=dat_sb[:, k, it_ * P:(it_ + 1) * P],
                                         rhs=t_sb[:, k, :],
                                         start=(k == 0), stop=(k == 3))
                    nc.vector.tensor_copy(out=s_sb[:, it_, :], in_=pu[:])
                    sq = swp.tile([P, EMB], F32, tag="sq")
                    nc.vector.tensor_tensor(out=sq[:], in0=s_sb[:, it_, :],
                                            in1=s_sb[:, it_, :], op=mybir.AluOpType.mult)
                    nr = swp.tile([P, 1], F32, tag="nr")
                    nc.vector.tensor_reduce(out=nr[:], in_=sq[:],
                                            axis=mybir.AxisListType.X,
                                            op=mybir.AluOpType.add)
                    nc.scalar.activation(out=nr[:], in_=nr[:],
                                         func=mybir.ActivationFunctionType.Sqrt)
                    nc.vector.tensor_scalar_max(out=nr[:], in0=nr[:], scalar1=1e-12)
                    nc.vector.reciprocal(out=nr[:], in_=nr[:])
                    nrm = swp.tile([P, EMB], F32, tag="nrm")
                    nc.vector.tensor_scalar_mul(out=nrm[:], in0=s_sb[:, it_, :],
                                                scalar1=nr[:, 0:1])
                    nc.vector.tensor_tensor(out=acc2_sb[:, it_, :], in0=acc2_sb[:, it_, :],
                                            in1=nrm[:], op=mybir.AluOpType.add)

            outt = scp.tile([P, 4, EMB], F32)
            for k in range(4):
                nc.vector.tensor_scalar_mul(out=outt[:, k, :], in0=acc2_sb[:, k, :],
                                            scalar1=1.0 / (LAYERS + 1))
                nc.sync.dma_start(out=result[k * P:(k + 1) * P, :], in_=outt[:, k, :])

    nc.compile()
    return nc


# --------------------------------------------------------------------------
# entry point
# --------------------------------------------------------------------------

_CACHE = {}


def _get_program(CPB, debug=False):
    key = (CPB, debug)
    if key not in _CACHE:
        _CACHE[key] = _build(CPB, debug)
    return _CACHE[key]


def kernel(**inputs):
    global LAST_EXEC_NS
    CPB, in_maps = _prep(inputs)
    nc = _get_program(CPB, debug=bool(int(os.environ.get("KDEBUG", "0"))))
    trace = TRACE
    if trace:
        try:
            import ntff_shim
            ntff_shim.install()
        except Exception:
            trace = False
    res = bass_utils.run_bass_kernel_spmd(
        nc, in_maps, core_ids=list(range(NCORES)), trace=trace)
    LAST_EXEC_NS = res.exec_time_ns
    kernel.last_results = res.results
    return res.results[0]["result"].astype(np.float32)
